# revision 20
# baseline (speedup 1.0000x reference)
"""KAN-SE (squeeze-excite with 2-layer KAN MLP) Trainium2 kernel.

Full-input contract: kernel(**inputs) takes the complete (32, 512, 64, 64)
batch plus KAN weights, shards the batch across 8 NeuronCores (4 samples
per core, data-parallel, weights replicated), and returns the full output.

Per-core device program (pure SPMD, no collectives), pipelined so the 16
DMA rings never wait on the KAN gate:
  - per-channel sums split in half per tile: Act reduces columns
    0:2048 (Copy + accumulator), DVE reduces 2048:4096 (tensor_scalar
    + accumulator) -- disjoint regions, so the halves run in parallel
    and the last tile's sum lands ~2.2us after its DMA completes
  - gate scales split g0/g1 on DVE (2x-mode multiply) and g2/g3 on
    Act (Copy activation with per-partition scale), placed at window
    start so stores flow while the next sample's chain runs
  - all DMA goes through the fast HWDGE path: every load issues from
    SP (the WAR dependency on the previous store's completion paces
    them, buffer depth 3 for the early tiles / 2 for the late ones),
    every store from Act right after the gate scales; the Pool engine
    is never used (its SWDGE descriptors move ~30% less bytes/ns and
    its Q7 elementwise path is ~17x slower than DVE)
  - KAN layer 1: fused Cox-de-Boor over all 4 channel groups via 3D
    access patterns; PE stationaries are host-duplicated to 128
    columns (bf16) so the layer-1 output lands replicated in both
    partition halves
  - KAN layer 2 exploits that replication: contraction packed as
    (hidden=64 x operand-pair) = 128 partitions -> 20 matmuls instead
    of 36; spline matmuls run before the silu-dependent ones so the
    activation path hides behind the PE chain
  - Act only ever runs Copy and Sigmoid (SiLU = x*sigmoid(x)), grouped
    so the activation-table switch is paid once per sample

x is read exactly once (SBUF-resident between mean and scale), so HBM
traffic is the 2x minimum: 8 MiB in + 8 MiB out per sample per core.
"""

import numpy as np
import ml_dtypes

BF16 = ml_dtypes.bfloat16

# ---- problem constants (hardcoded per contract; do not read spec/reference) ----
B, C, H, W = 32, 512, 64, 64
HIDDEN = 64            # max(16, 512 // 8)
KB = 8                 # GRID_SIZE + SPLINE_ORDER = 5 + 3
NCORES = 8
NS = B // NCORES       # samples per core = 4
NG = C // 128          # channel groups of 128 = 4
HWPIX = H * W          # 4096
HALF = HWPIX // 2
NPAIR = 5              # layer-2 operand pairs: (b0,b1)..(b6,b7), (base,-)

_GT_OFF = {"G0": 0, 1: (12, 22), 2: (32, 41), 3: (50, 58)}
_GT_W = 66
_F_OFF = {"G": 0, (1, "A"): 48, (1, "C"): 88, (2, "A"): 128, (2, "C"): 164,
          (3, "A"): 200, (3, "C"): 232}
_F_W = 264


def _grid_tables2(grid_row: np.ndarray):
    g = np.asarray(grid_row, np.float64)
    assert g.shape == (12,)
    h = g[1] - g[0]
    tab = np.zeros((_GT_W,), np.float64)
    tab[0:12] = g
    rs = {}
    for k in (1, 2, 3):
        w = 11 - k
        aoff, coff = _GT_OFF[k]
        tab[aoff:aoff + w] = -g[:w]
        tab[coff:coff + w] = g[k + 1:12]
        rs[k] = float(np.float32(1.0 / (k * h)))
    full = np.tile(tab.astype(np.float32)[None, :], (128, 1))
    return np.ascontiguousarray(full), rs


def _grid_tables1(grid_row: np.ndarray):
    """Fused layer-1 (128, 264) table with 1/(k h) folded into the knots."""
    g = np.asarray(grid_row, np.float64)
    assert g.shape == (12,)
    h = g[1] - g[0]
    tab = np.zeros((_F_W,), np.float64)
    for gb in range(NG):
        tab[12 * gb: 12 * gb + 12] = g
    rs = {}
    for k in (1, 2, 3):
        w = 11 - k
        r = 1.0 / (k * h)
        rs[k] = float(np.float32(r))
        for gb in range(NG):
            a0 = _F_OFF[(k, "A")] + w * gb
            c0 = _F_OFF[(k, "C")] + w * gb
            tab[a0:a0 + w] = -g[:w] * r
            tab[c0:c0 + w] = g[k + 1:12] * r
    full = np.tile(tab.astype(np.float32)[None, :], (128, 1))
    return np.ascontiguousarray(full), rs


def _host_prep(inputs):
    """Rearrange weights into the SBUF layouts the device program uses."""
    f32 = np.float32
    base_w1 = np.asarray(inputs["base_w1"], f32)      # (64, 512)
    spline_w1 = np.asarray(inputs["spline_w1"], f32)  # (64, 512, 8)
    scaler1 = np.asarray(inputs["scaler1"], f32)      # (64, 512)
    base_w2 = np.asarray(inputs["base_w2"], f32)      # (512, 64)
    spline_w2 = np.asarray(inputs["spline_w2"], f32)  # (512, 64, 8)
    scaler2 = np.asarray(inputs["scaler2"], f32)      # (512, 64)

    # Layer-1 stationaries, duplicated to 128 output columns so ps1 lands
    # replicated in both partition halves.
    w1 = base_w1.reshape(HIDDEN, NG, 128)             # (64, g, p)
    w1d = np.concatenate([w1, w1], axis=0)            # (128c, g, p)
    w1d = np.ascontiguousarray(
        w1d.transpose(2, 1, 0).reshape(128, NG * 128).astype(BF16))
    sw = (spline_w1 * scaler1[:, :, None]).reshape(HIDDEN, NG, 128, KB)
    swd = np.concatenate([sw, sw], axis=0)            # (128c, g, p, k)
    sw1d = np.ascontiguousarray(
        swd.transpose(2, 1, 3, 0).reshape(128, NG * KB * 128).astype(BF16))

    # Layer-2 pair-packed stationaries: contraction index q = i + 64*h.
    # W2p[q, (og*5+pair)*128 + p] = W_{j(pair,h)}[128og + p, i]
    sw2s = spline_w2 * scaler2[:, :, None]            # (512, 64, 8)
    w2p = np.zeros((128, NG * NPAIR * 128), f32)
    for og in range(NG):
        osl = slice(128 * og, 128 * (og + 1))
        for pair in range(4):
            col = (og * NPAIR + pair) * 128
            w2p[0:64, col:col + 128] = sw2s[osl, :, 2 * pair].T
            w2p[64:128, col:col + 128] = sw2s[osl, :, 2 * pair + 1].T
        col = (og * NPAIR + 4) * 128
        w2p[0:64, col:col + 128] = base_w2[osl, :].T  # silu2 pair; h=1 stays 0
    w2p = np.ascontiguousarray(w2p.astype(BF16))

    gt1, rs1 = _grid_tables1(np.asarray(inputs["grid1"], f32)[0])
    gt2, rs2 = _grid_tables2(np.asarray(inputs["grid2"], f32)[0])

    tensors = {"w1d": w1d, "sw1d": sw1d, "w2p": w2p, "gt1": gt1, "gt2": gt2}
    return tensors, rs1, rs2


def _emit_bsplines2(nc, mybir, pool, gt_sb, x_ap, out_ap, p, rs):
    """Layer-2 cubic B-spline bases (one value per partition) -> out_ap (p, 8)."""
    f32 = mybir.dt.float32
    Alu = mybir.AluOpType
    ge = pool.tile([128, 12], f32, tag="ge2", bufs=2)
    nc.vector.tensor_scalar(
        out=ge[:p], in0=gt_sb[:p, 0:12], scalar1=x_ap, scalar2=None, op0=Alu.is_le
    )
    bprev = pool.tile([128, 11], f32, tag="b02", bufs=2)
    nc.vector.tensor_tensor(bprev[:p], ge[:p, 0:11], ge[:p, 1:12], Alu.subtract)
    for k in (1, 2, 3):
        w = 11 - k
        aoff, coff = _GT_OFF[k]
        a_t = pool.tile([128, 10], f32, tag="bsA2", bufs=2)
        c_t = pool.tile([128, 10], f32, tag="bsC2", bufs=2)
        nc.vector.tensor_scalar(
            out=a_t[:p, :w], in0=gt_sb[:p, aoff:aoff + w], scalar1=x_ap,
            scalar2=rs[k], op0=Alu.add, op1=Alu.mult,
        )
        nc.vector.tensor_scalar(
            out=c_t[:p, :w], in0=gt_sb[:p, coff:coff + w], scalar1=x_ap,
            scalar2=rs[k], op0=Alu.subtract, op1=Alu.mult,
        )
        if k < 3:
            bnext = pool.tile([128, 10], f32, tag="bn2", bufs=2)
            outp = bnext[:p, :w]
        else:
            outp = out_ap
        nc.vector.tensor_tensor(c_t[:p, :w], c_t[:p, :w], bprev[:p, 1:w + 1], Alu.mult)
        nc.vector.tensor_tensor(outp, a_t[:p, :w], bprev[:p, 0:w], Alu.mult)
        nc.vector.tensor_tensor(outp, outp, c_t[:p, :w], Alu.add)
        if k < 3:
            bprev = bnext
    return out_ap


def _build_nc(rs1, rs2):
    import concourse.bacc as bacc
    import concourse.bass as bass  # noqa: F401
    import concourse.mybir as mybir
    from concourse.tile import TileContext

    f32 = mybir.dt.float32
    bf16 = mybir.dt.bfloat16
    Alu = mybir.AluOpType
    Act = mybir.ActivationFunctionType

    nc = bacc.Bacc("TRN2", target_bir_lowering=False)
    x_d = nc.declare_dram_parameter("x", [NS, C, H, W], f32, isOutput=False)
    w1d_d = nc.declare_dram_parameter("w1d", [128, NG * 128], bf16, isOutput=False)
    sw1d_d = nc.declare_dram_parameter("sw1d", [128, NG * KB * 128], bf16, isOutput=False)
    w2p_d = nc.declare_dram_parameter("w2p", [128, NG * NPAIR * 128], bf16, isOutput=False)
    gt1_d = nc.declare_dram_parameter("gt1", [128, _F_W], f32, isOutput=False)
    gt2_d = nc.declare_dram_parameter("gt2", [128, _GT_W], f32, isOutput=False)
    y_d = nc.declare_dram_parameter("y", [NS, C, H, W], f32, isOutput=True)

    def g3(ap):
        return ap.rearrange("p (g c) -> p g c", g=NG)

    with TileContext(nc) as tc:
        with (
            tc.tile_pool(name="consts", bufs=1) as cpool,
            tc.tile_pool(name="xa", bufs=3) as xapool,
            tc.tile_pool(name="xb", bufs=2) as xbpool,
            tc.tile_pool(name="small", bufs=3) as spool,
            tc.tile_pool(name="bspl", bufs=2) as bpool,
            tc.tile_pool(name="psum", bufs=1, space="PSUM") as ppool,
        ):
            xts = [None] * NS

            def load_tile(n, eng, g):
                if xts[n] is None:
                    xts[n] = [None] * NG
                pool = xapool if g < 2 else xbpool
                xt = pool.tile([128, HWPIX], f32, tag=f"xt{g}", name=f"xt{g}_{n}")
                src = x_d[n, 128 * g:128 * (g + 1)].rearrange("p h w -> p (h w)")
                eng.dma_start(xt[:], src)
                xts[n][g] = xt

            w1d_sb = cpool.tile([128, NG * 128], bf16)
            nc.sync.dma_start(w1d_sb[:], w1d_d[:, :])
            sw1d_sb = cpool.tile([128, NG * KB * 128], bf16)
            nc.sync.dma_start(sw1d_sb[:], sw1d_d[:, :])
            w2p_sb = cpool.tile([128, NG * NPAIR * 128], bf16)
            nc.sync.dma_start(w2p_sb[:], w2p_d[:, :])
            gt1_sb = cpool.tile([128, _F_W], f32)
            nc.sync.dma_start(gt1_sb[:], gt1_d[:, :])
            gt2_sb = cpool.tile([128, _GT_W], f32)
            nc.sync.dma_start(gt2_sb[:], gt2_d[:, :])
            zeros12 = cpool.tile([128, 12], f32)
            nc.vector.memset(zeros12[:], 0.0)
            junkA = cpool.tile([128, HALF], f32)   # Act half-reduce out
            junkB = cpool.tile([128, HALF], f32)   # DVE half-reduce out

            # Pre-touch const tiles so single-wait-slot consumers never
            # need a DMA wait of their own.
            touch = cpool.tile([128, 8], f32)
            for i, ct in enumerate((w1d_sb, sw1d_sb, w2p_sb, gt1_sb, gt2_sb)):
                nc.vector.tensor_copy(touch[:, i:i + 1], ct[:, 0:1])
            pt_ps = ppool.tile([128, 1], f32, tag="l1p0")
            for ct in (w1d_sb, sw1d_sb, w2p_sb):
                nc.tensor.matmul(pt_ps[0:1, 0:1], ct[:, 0:1], ct[:, 0:1],
                                 start=True, stop=True)

            gt48 = g3(gt1_sb[:, 0:48])
            gA = {k: g3(gt1_sb[:, _F_OFF[(k, "A")]:_F_OFF[(k, "A")] + NG * (11 - k)])
                  for k in (1, 2, 3)}
            gC = {k: g3(gt1_sb[:, _F_OFF[(k, "C")]:_F_OFF[(k, "C")] + NG * (11 - k)])
                  for k in (1, 2, 3)}

            for sn in range(NS):
                for g in range(NG):
                    load_tile(sn, nc.sync, g)

            gates = [None] * NS
            inv = 1.0 / HWPIX

            for n in range(NS + 1):
                m = n - 1
                # ---- scales for sample m at window start:
                #      g0/g1 on DVE (2x-mode mult), g2/g3 on Act ----
                if n >= 1:
                    gate = gates[m]
                    for g in (0, 1):
                        nc.vector.tensor_scalar(
                            out=xts[m][g][:], in0=xts[m][g][:],
                            scalar1=gate[:, g:g + 1], scalar2=None, op0=Alu.mult,
                        )
                    for g in (2, 3):
                        nc.scalar.activation(xts[m][g][:], xts[m][g][:], Act.Copy,
                                             scale=gate[:, g:g + 1])
                    for g in range(NG):
                        dst = y_d[m, 128 * g:128 * (g + 1)].rearrange(
                            "p h w -> p (h w)")
                        nc.scalar.dma_start(dst, xts[m][g][:])
                if n == NS:
                    break

                # ---- half reduces: Act cols 0:HALF, DVE cols HALF: ----
                sTa = spool.tile([128, NG], f32, tag="sTa")
                sTb = spool.tile([128, NG], f32, tag="sTb")
                for g in range(NG):
                    nc.scalar.activation(junkA[:], xts[n][g][:, 0:HALF], Act.Copy,
                                         accum_out=sTa[:, g:g + 1])
                for g in range(NG):
                    nc.vector.tensor_scalar(
                        out=junkB[:], in0=xts[n][g][:, HALF:], scalar1=1.0,
                        scalar2=0.0, op0=Alu.mult, op1=Alu.add,
                        accum_out=sTb[:, g:g + 1],
                    )

                sT = spool.tile([128, NG], f32, tag="sT")
                nc.vector.tensor_tensor(sT[:], sTa[:], sTb[:], Alu.add)

                # sigs = sigmoid(mean); silu1 = mean * sigs (bf16 for PE)
                sigs = spool.tile([128, NG], f32, tag="sigs")
                nc.scalar.activation(sigs[:], sT[:], Act.Sigmoid, scale=inv)
                silu1 = spool.tile([128, NG], bf16, tag="silu1")
                nc.vector.scalar_tensor_tensor(
                    out=silu1[:], in0=sT[:], scalar=inv, in1=sigs[:],
                    op0=Alu.mult, op1=Alu.mult,
                )

                # ---- fused layer-1 Cox-de-Boor ----
                xrep = bpool.tile([128, NG * 12], f32, tag="xrep")
                for g in range(NG):
                    nc.vector.tensor_scalar(
                        out=xrep[:, 12 * g:12 * g + 12], in0=zeros12[:],
                        scalar1=sT[:, g:g + 1], scalar2=inv, op0=Alu.add, op1=Alu.mult,
                    )
                xrv = g3(xrep[:])
                ge = bpool.tile([128, NG * 12], f32, tag="geF")
                nc.vector.tensor_tensor(ge[:], gt48, xrv, Alu.is_le)
                bprev = bpool.tile([128, NG * 11], f32, tag="b0F")
                gev = g3(ge[:])
                nc.vector.tensor_tensor(g3(bprev[:]), gev[:, :, 0:11],
                                        gev[:, :, 1:12], Alu.subtract)
                bpv = g3(bprev[:])
                bf = spool.tile([128, NG * KB], f32, tag="bf")
                for k in (1, 2, 3):
                    w = 11 - k
                    a_t = bpool.tile([128, NG * 10], f32, tag="bsAF")
                    c_t = bpool.tile([128, NG * 10], f32, tag="bsCF")
                    av = g3(a_t[:, :NG * w])
                    cv = g3(c_t[:, :NG * w])
                    nc.vector.scalar_tensor_tensor(
                        out=av, in0=xrv[:, :, 0:w], scalar=rs1[k], in1=gA[k],
                        op0=Alu.mult, op1=Alu.add,
                    )
                    nc.vector.scalar_tensor_tensor(
                        out=cv, in0=xrv[:, :, 0:w], scalar=-rs1[k], in1=gC[k],
                        op0=Alu.mult, op1=Alu.add,
                    )
                    if k < 3:
                        bnext = bpool.tile([128, NG * 10], f32, tag="bnF")
                        outp = g3(bnext[:, :NG * w])
                    else:
                        outp = g3(bf[:])
                    nc.vector.tensor_tensor(cv, cv, bpv[:, :, 1:w + 1], Alu.mult)
                    nc.vector.tensor_tensor(outp, av, bpv[:, :, 0:w], Alu.mult)
                    nc.vector.tensor_tensor(outp, outp, cv, Alu.add)
                    if k < 3:
                        bprev, bpv = bnext, g3(bnext[:, :NG * w])
                bfb = spool.tile([128, NG * KB], bf16, tag="bfb")
                nc.vector.tensor_copy(bfb[:], bf[:])

                # ---- layer-1 matmuls: 4 independent PSUM chains,
                #      spline terms first so silu1 hides behind them ----
                pss = [ppool.tile([128, 1], f32, tag=f"l1p{g}", name=f"l1p{g}")
                       for g in range(NG)]
                for k in range(KB):
                    for g in range(NG):
                        col = 128 * (KB * g + k)
                        nc.tensor.matmul(pss[g][:], sw1d_sb[:, col:col + 128],
                                         bfb[:, KB * g + k:KB * g + k + 1],
                                         start=(k == 0), stop=False)
                for g in range(NG):
                    nc.tensor.matmul(pss[g][:], w1d_sb[:, 128 * g:128 * (g + 1)],
                                     silu1[:, g:g + 1], start=False, stop=True)
                h1 = spool.tile([128, 1], f32, tag="h1")
                nc.vector.tensor_copy(h1[:], pss[0][:])
                nc.vector.tensor_tensor(h1[:], h1[:], pss[1][:], Alu.add)
                nc.vector.tensor_tensor(h1[:], h1[:], pss[2][:], Alu.add)
                nc.vector.tensor_tensor(h1[:], h1[:], pss[3][:], Alu.add)

                # ---- inter-layer: t1 = silu(h1), bases2, v' packing ----
                sig1 = spool.tile([128, 1], f32, tag="sig1")
                nc.scalar.activation(sig1[:], h1[:], Act.Sigmoid)
                t1 = spool.tile([128, 1], f32, tag="t1")
                nc.vector.tensor_tensor(t1[:], h1[:], sig1[:], Alu.mult)
                sig2 = spool.tile([128, 1], f32, tag="sig2")
                nc.scalar.activation(sig2[:], t1[:], Act.Sigmoid)
                b2f = spool.tile([128, KB], f32, tag="b2f")
                _emit_bsplines2(nc, mybir, bpool, gt2_sb, t1[:, 0:1], b2f[:],
                                128, rs2)
                vp = spool.tile([128, NPAIR], bf16, tag="vp")
                for pair in range(4):
                    nc.vector.tensor_copy(vp[0:64, pair:pair + 1],
                                          b2f[0:64, 2 * pair:2 * pair + 1])
                    nc.vector.tensor_copy(vp[64:128, pair:pair + 1],
                                          b2f[64:128, 2 * pair + 1:2 * pair + 2])
                # silu2 written straight into the base-pair column
                nc.vector.tensor_tensor(vp[:, 4:5], t1[:], sig2[:], Alu.mult)

                # ---- layer-2 matmuls: 20, basis pairs first ----
                ps2 = [ppool.tile([128, 1], f32, tag=f"l2p{og}", name=f"l2p{og}")
                       for og in range(NG)]
                for pair in range(4):
                    for og in range(NG):
                        col = 128 * (og * NPAIR + pair)
                        nc.tensor.matmul(ps2[og][:], w2p_sb[:, col:col + 128],
                                         vp[:, pair:pair + 1],
                                         start=(pair == 0), stop=False)
                for og in range(NG):
                    col = 128 * (og * NPAIR + 4)
                    nc.tensor.matmul(ps2[og][:], w2p_sb[:, col:col + 128],
                                     vp[:, 4:5], start=False, stop=True)

                gate = spool.tile([128, NG], f32, tag="gate")
                for og in range(NG):
                    nc.scalar.activation(gate[:, og:og + 1], ps2[og][:], Act.Sigmoid)
                gates[n] = gate
    nc.compile()
    return nc


def _run(inputs, trace=False):
    from concourse.bass_utils import run_bass_kernel_spmd

    x = np.ascontiguousarray(np.asarray(inputs["x"], np.float32))
    assert x.shape == (B, C, H, W), x.shape
    tensors, rs1, rs2 = _host_prep(inputs)
    nc = _build_nc(rs1, rs2)
    in_maps = []
    for c in range(NCORES):
        m = {"x": np.ascontiguousarray(x[NS * c:NS * (c + 1)])}
        m.update(tensors)
        in_maps.append(m)
    res = run_bass_kernel_spmd(
        nc, in_maps, core_ids=list(range(NCORES)), trace=trace
    )
    out = np.concatenate([res.results[c]["y"] for c in range(NCORES)], axis=0)
    return out, res


def kernel(**inputs) -> np.ndarray:
    return _run(inputs)[0]
